# revision 30
# baseline (speedup 1.0000x reference)
"""Causal self-attention (B=4, S=2048, D=1024, H=16) on 8 trn2 cores.

Sharding: core c = 2*b + g  (b = batch 0..3, g = head-group 0..1, 8 heads/group).
Each core computes, for its batch element and its 8 heads:
    qkv -> causal attention -> y @ w_proj[rows of its head group]
The two head-group partial outputs per batch are summed on the host.

Device layouts:
    xT [D, S] bf16    x[b] transposed (contraction dim on partitions)
    wq/wk/wv [D, 512] bf16   w_qkv column slices for the group
    wp [512, D] bf16  w_proj row slice
Matmul inputs run in bf16 (halves DMA, enables fast-weight-load); all
matmul accumulation is fp32 in PSUM, so softmax denominators and the
projection sums keep fp32 accuracy.  Scores are computed transposed
([sk, sq]) so exp(p) feeds the AV matmul directly as the moving
operand; an all-ones column appended to V gives the softmax denominator
for free (row 64 of the AV psum).

Perf structure (v6):
  - dummy warmup matmuls at t=0 bridge the input-DMA window so the PE
    HAM clock-gate opens before real work arrives; xT is loaded in
    s-slices so the V GEMM starts as soon as the first slice lands.
  - attn slot order: j-major for j=0,1 then (2,t)/(3,t) interleaved,
    so each 512-query block finishes mid-phase and its projection can
    overlap later attention instead of piling into a serial tail.
  - qk GEMMs ride two slots ahead as fillers inside attn tiles so the
    PE FIFO always has ready matmuls while ACT runs exp; the two
    heads' scores matmuls are interleaved per chunk so their 64-row
    LDWEIGHTS overlap the other head's in-flight matmul.
  - softmax normalization is deferred to the projection: the attn
    epilogue stores unnormalized y^T and the denominator row l; per
    512-query block l is reciprocal'd in a compact [8,512] layout and
    broadcast to rl [128, 4, 512]; proj inputs are scaled by rl two
    slots later, keeping the chain off the critical path.
"""

import numpy as np
import ml_dtypes

import concourse.mybir as mybir
import concourse.tile as tile
from concourse import bacc
from concourse.bass_utils import run_bass_kernel_spmd

P = 128
D = 1024
KD = D // P          # 8 contraction chunks
GCOLS = 512          # qkv cols per head group
HG = 8               # heads per core
HD = 64
NJ = 4               # head-pair col tiles (2 heads x 64 = 128)
SQT = 512            # sq tile (matmul moving dim)
F32 = mybir.dt.float32
F32R = mybir.dt.float32r
BF16 = mybir.dt.bfloat16

TRACE = False
PPOOL_BUFS = 3
SC_BUFS = 3
ST_BUFS = 4
QKT_BUFS = 3
YP_BUFS = 2
WS_BUFS = 2
PJ_BUFS = 5
WARM_MMS = 56
TRACE_KWARGS = {}


def _r(ap):
    return ap.bitcast(F32R)


def build_nc(S=2048):
    NT4 = S // SQT       # sq tiles of 512
    NT16 = S // P        # s chunks of 128
    nc = bacc.Bacc("TRN2", target_bir_lowering=False, debug=False)

    xT = nc.dram_tensor("xT", [D, S], BF16, kind="ExternalInput").ap()
    wq = nc.dram_tensor("wq", [D, GCOLS], BF16, kind="ExternalInput").ap()
    wk = nc.dram_tensor("wk", [D, GCOLS], BF16, kind="ExternalInput").ap()
    wv = nc.dram_tensor("wv", [D, GCOLS], BF16, kind="ExternalInput").ap()
    wp = nc.dram_tensor("wp", [GCOLS, D], BF16, kind="ExternalInput").ap()
    mk = nc.dram_tensor("mk", [P, 4, SQT], BF16, kind="ExternalInput").ap()
    out = nc.dram_tensor("out", [S, D], BF16, kind="ExternalOutput").ap()

    with tile.TileContext(nc) as tc:
        with (
            tc.tile_pool(name="persist", bufs=1) as persist,
            tc.tile_pool(name="qkt", bufs=QKT_BUFS) as qkt,
            tc.tile_pool(name="ppool", bufs=PPOOL_BUFS) as ppool,
            tc.tile_pool(name="stpool", bufs=ST_BUFS) as stpool,
            tc.tile_pool(name="wvpool", bufs=1) as wvpool,
            tc.tile_pool(name="qkv_in", bufs=1) as qkv_in,
            tc.tile_pool(name="wstream", bufs=WS_BUFS) as wstream,
            tc.tile_pool(name="late", bufs=1) as late,
            tc.tile_pool(name="projin", bufs=PJ_BUFS) as projin,
            tc.tile_pool(name="outst", bufs=PJ_BUFS) as outst,
            tc.tile_pool(name="rlpool", bufs=2) as rlpool,
            tc.tile_pool(name="ltpool", bufs=2) as ltpool,
            tc.tile_pool(name="dram", bufs=1, space="DRAM") as drampool,
            tc.tile_pool(name="ps_sc", bufs=SC_BUFS, space="PSUM") as ps_sc,
            tc.tile_pool(name="ps_y", bufs=YP_BUFS, space="PSUM") as ps_y,
        ):
            V = persist.tile([P, NT16, HG, HD + 1], BF16)
            MK = persist.tile([P, 4, SQT], BF16)
            ld = drampool.tile([HG, S], BF16)   # softmax denominators
            rd = drampool.tile([HG, S], BF16)   # their reciprocals
            yd = drampool.tile([GCOLS, S], BF16)
            rdr = rd.rearrange("(one h) s -> one h s", one=1)

            # warmup source: reused later as the all-ones column of V
            onesrow = persist.tile([P, NT16 * HG], F32)
            nc.vector.memset(onesrow, 1.0)
            for _ in range(WARM_MMS // 4):
                wps = ps_sc.tile([P, P], F32, name="wps", tag="sc")
                for i in range(4):
                    nc.tensor.matmul(
                        wps[0:4, :],
                        lhsT=_r(onesrow[:, 0:4]),
                        rhs=_r(onesrow[:, 0:P]),
                        start=(i == 0),
                        stop=(i == 3),
                    )

            wvs = wvpool.tile([P, KD, GCOLS], BF16, tag="wv")
            xTs = qkv_in.tile([P, KD, S], BF16)

            # ---- input loads: wv + xT slice 0 split across queues so the
            # V GEMM can start ASAP; MK last (not needed until attn) ----
            wvr = wv.rearrange("(k p) c -> p k c", p=P)
            xTr = xT.rearrange("(k p) s -> p k s", p=P)
            nc.sync.dma_start(out=wvs[:, 0:4, :], in_=wvr[:, 0:4, :])
            nc.scalar.dma_start(out=xTs[:, 0:4, 0:SQT], in_=xTr[:, 0:4, 0:SQT])
            nc.sync.dma_start(out=wvs[:, 4:KD, :], in_=wvr[:, 4:KD, :])
            nc.gpsimd.dma_start(out=xTs[:, 4:KD, 0:SQT], in_=xTr[:, 4:KD, 0:SQT])
            for t4, eng in ((1, nc.sync), (2, nc.gpsimd), (3, nc.scalar)):
                eng.dma_start(
                    out=xTs[:, :, t4 * SQT : (t4 + 1) * SQT],
                    in_=xTr[:, :, t4 * SQT : (t4 + 1) * SQT],
                )
            nc.sync.dma_start(out=MK, in_=mk)
            nc.vector.tensor_copy(
                out=V[:, :, :, HD : HD + 1],
                in_=onesrow.rearrange("p (t h one) -> p t h one", t=NT16, one=1),
            )

            # ---- V = x @ wv  (natural [s, vcol] layout) ----
            for t in range(NT16):
                ps = ps_sc.tile([P, GCOLS], F32, name="ps_v", tag="sc")
                for k in range(KD):
                    nc.tensor.matmul(
                        ps,
                        lhsT=xTs[:, k, t * P : (t + 1) * P],
                        rhs=wvs[:, k, :],
                        start=(k == 0),
                        stop=(k == KD - 1),
                    )
                nc.scalar.copy(
                    out=V[:, t, :, 0:HD],
                    in_=ps.rearrange("p (h d) -> p h d", h=HG),
                )

            # WP reuses the wvs slot (ring dep: waits for the V GEMM), so
            # its DMA lands during the first attn slots.
            WP = wvpool.tile([P, NJ, D], BF16, tag="wv", name="WP")
            nc.sync.dma_start(out=WP, in_=wp.rearrange("(j p) d -> p j d", p=P))
            Y3 = late.tile([P, S], BF16)
            ydr = yd.rearrange("(j p) s -> p j s", p=P)
            rl_sb = {}

            qT_sb = {}
            kT_sb = {}
            wq_sb = {}
            wk_sb = {}

            def qk_w_load(j):
                wqj = wstream.tile([P, KD, P], BF16, tag="wqj")
                wkj = wstream.tile([P, KD, P], BF16, tag="wkj")
                nc.sync.dma_start(
                    out=wqj,
                    in_=wq[:, j * P : (j + 1) * P].rearrange("(k p) c -> p k c", p=P),
                )
                nc.sync.dma_start(
                    out=wkj,
                    in_=wk[:, j * P : (j + 1) * P].rearrange("(k p) c -> p k c", p=P),
                )
                wq_sb[j] = wqj
                wk_sb[j] = wkj
                qT_sb[j] = qkt.tile([P, S], BF16, name="qTj", tag="qTj")
                kT_sb[j] = qkt.tile([P, S], BF16, name="kTj", tag="kTj")

            def qk_dest(j, t, which):
                """One dest (q or k) of head-pair j, s-tile t: 8 matmuls."""
                wsb, dest = (
                    (wq_sb[j], qT_sb[j]) if which == "q" else (wk_sb[j], kT_sb[j])
                )
                ps = ps_sc.tile([P, SQT], F32, name="ps_qk", tag="sc")
                for k in range(KD):
                    nc.tensor.matmul(
                        ps,
                        lhsT=wsb[:, k, :],
                        rhs=xTs[:, k, t * SQT : (t + 1) * SQT],
                        start=(k == 0),
                        stop=(k == KD - 1),
                    )
                nc.vector.tensor_copy(out=dest[:, t * SQT : (t + 1) * SQT], in_=ps)

            def attn_tile(j, t, fillers=()):
                """Scores+softmax+AV for heads (2j, 2j+1) on sq tile t.

                Software-pipelined: AV of group g is emitted after the
                scores+exp of group g+1 so PE has work while ACT runs."""
                qTj = qT_sb[j]
                kTj = kT_sb[j]
                nch = 4 * t + 4  # causal sk chunks of 128
                yps = {}
                for hi in (0, 1):
                    h = 2 * j + hi
                    yps[h] = ps_y.tile([HD + 1, SQT], F32, name="yps", tag="yps")

                def chunk_off(c):
                    # exact causal column offset within the sq tile (bf16
                    # matmuls run full-rate at any width)
                    if c < 4 * t:
                        return 0
                    return (c - 4 * t) * P

                def emit_scores_exp(g):
                    w = min(2, nch - g)
                    offs = [chunk_off(g + ci) for ci in range(w)]
                    pts = {}
                    scs = {}
                    for hi in (0, 1):
                        scs[hi] = ps_sc.tile([P, 2 * SQT], F32, name="sc", tag="sc")
                    # interleave the two heads' matmuls: their 64-row
                    # groups (h0/h64) don't conflict, so each LDWEIGHTS
                    # overlaps the other head's in-flight matmul.
                    for ci in range(w):
                        c = g + ci
                        off = offs[ci]
                        for hi in (0, 1):
                            base = HD * hi
                            nc.tensor.matmul(
                                scs[hi][:, ci * SQT + off : (ci + 1) * SQT],
                                lhsT=kTj[base : base + HD, c * P : (c + 1) * P],
                                rhs=qTj[
                                    base : base + HD,
                                    t * SQT + off : (t + 1) * SQT,
                                ],
                                start=True,
                                stop=True,
                            )
                    for hi in (0, 1):
                        h = 2 * j + hi
                        sc = scs[hi]
                        p = ppool.tile([P, 2 * SQT], BF16, name="pexp")
                        # one exp call spanning all chunks of the group; for
                        # diag groups this also exps the dead strip between
                        # chunk windows (stale psum) — it is never read.
                        nc.scalar.activation(
                            out=p[:, offs[0] : w * SQT],
                            in_=sc[:, offs[0] : w * SQT],
                            func=mybir.ActivationFunctionType.Exp,
                            scale=0.125,
                        )
                        for ci in range(w):
                            c = g + ci
                            if c >= 4 * t:  # diagonal: zero non-causal
                                m = c - 4 * t
                                off = offs[ci]
                                wd = SQT - off
                                psl = p[:, ci * SQT + off : (ci + 1) * SQT]
                                if hi == 0:
                                    nc.vector.tensor_mul(
                                        psl, psl, MK[:, m, off:SQT]
                                    )
                                else:
                                    nc.gpsimd.affine_select(
                                        out=psl,
                                        in_=psl,
                                        compare_op=mybir.AluOpType.is_ge,
                                        fill=0.0,
                                        base=off - P * m,
                                        channel_multiplier=-1,
                                        pattern=[[1, wd]],
                                    )
                        pts[h] = p
                    return pts

                def emit_av(g, pts):
                    w = min(2, nch - g)
                    for hi in (0, 1):
                        h = 2 * j + hi
                        for ci in range(w):
                            c = g + ci
                            off = chunk_off(c)
                            nc.tensor.matmul(
                                yps[h][:, off:SQT],
                                lhsT=V[:, c, h, :],
                                rhs=pts[h][:, ci * SQT + off : (ci + 1) * SQT],
                                start=(c == 0),
                                stop=(c == nch - 1),
                            )

                fill_iter = iter(fillers)
                prev = None
                for g in range(0, nch, 2):
                    pts = emit_scores_exp(g)
                    if prev is not None:
                        emit_av(*prev)
                    f = next(fill_iter, None)
                    if f is not None:
                        f()
                    prev = (g, pts)
                emit_av(*prev)
                for f in fill_iter:
                    f()

                for hi in (0, 1):
                    h = 2 * j + hi
                    st = stpool.tile([HD + 1, SQT], BF16, name="st")
                    nc.vector.tensor_copy(out=st, in_=yps[h])
                    nc.gpsimd.dma_start(
                        out=ld[h : h + 1, t * SQT : (t + 1) * SQT],
                        in_=st[HD : HD + 1, :],
                    )
                    if j == NJ - 1:
                        ydst = Y3[
                            HD * hi : HD * (hi + 1), t * SQT : (t + 1) * SQT
                        ]
                    else:
                        ydst = yd[
                            j * P + HD * hi : j * P + HD * (hi + 1),
                            t * SQT : (t + 1) * SQT,
                        ]
                    nc.sync.dma_start(out=ydst, in_=st[0:HD, :])

            def l_recip(T):
                """Reciprocal of the 8 heads' denominators for sq block T
                in the compact [8, 512] layout, staged back to DRAM."""
                lt = ltpool.tile([HG, SQT], BF16, name="lt")
                nc.gpsimd.dma_start(out=lt, in_=ld[:, T * SQT : (T + 1) * SQT])
                rt = ltpool.tile([HG, SQT], BF16, name="rt")
                with nc.allow_low_precision(reason="bf16 1/l"):
                    nc.vector.reciprocal(out=rt, in_=lt)
                nc.gpsimd.dma_start(out=rd[:, T * SQT : (T + 1) * SQT], in_=rt)

            def rl_build(T):
                """Broadcast 1/l to rl [128, NJ, SQT] (partition = y-col)."""
                rl = rlpool.tile([P, NJ, SQT], BF16, name="rl")
                for hi in (0, 1):
                    nc.gpsimd.dma_start(
                        out=rl[hi * HD : (hi + 1) * HD, :, :],
                        in_=rdr[
                            0:1, hi : HG : 2, T * SQT : (T + 1) * SQT
                        ].to_broadcast([HD, NJ, SQT]),
                    )
                rl_sb[T] = rl

            def proj_tile(tp):
                T, q = divmod(tp, 4)
                rl = rl_sb[T]
                rls = rl[:, :, q * P : (q + 1) * P]
                yt = projin.tile([P, NJ - 1, P], BF16, name="yt")
                nc.scalar.dma_start(
                    out=yt, in_=ydr[:, 0 : NJ - 1, tp * P : (tp + 1) * P]
                )
                nc.vector.tensor_mul(yt, yt, rls[:, 0 : NJ - 1, :])
                y3s = Y3[:, tp * P : (tp + 1) * P]
                nc.vector.tensor_mul(y3s, y3s, rls[:, NJ - 1, :])
                for n in range(D // SQT):
                    pp = ps_sc.tile([P, SQT], F32, name="pp", tag="sc")
                    for j in range(NJ):
                        lhsT = yt[:, j, :] if j < NJ - 1 else y3s
                        nc.tensor.matmul(
                            pp,
                            lhsT=lhsT,
                            rhs=WP[:, j, n * SQT : (n + 1) * SQT],
                            start=(j == 0),
                            stop=(j == NJ - 1),
                        )
                    ot = outst.tile([P, SQT], BF16, name="ot")
                    nc.vector.tensor_copy(out=ot, in_=pp)
                    nc.sync.dma_start(
                        out=out[tp * P : (tp + 1) * P, n * SQT : (n + 1) * SQT],
                        in_=ot,
                    )

            # ---- slot schedule ----
            A = (
                [(0, t) for t in range(NT4)]
                + [(1, t) for t in range(NT4)]
                + [(2, 0), (3, 0), (2, 1), (3, 1), (2, 2), (3, 2), (2, 3), (3, 3)]
            )
            slot_fillers = [[] for _ in A]

            def add_qk(s, jq, tq):
                slot_fillers[s].append(lambda a=jq, b=tq: qk_dest(a, b, "q"))
                slot_fillers[s].append(lambda a=jq, b=tq: qk_dest(a, b, "k"))

            # qk deliveries ride ~two slots ahead of their attn slot
            add_qk(0, *A[1])
            add_qk(0, *A[2])
            for i in range(1, 14):
                add_qk(i, *A[i + 2])
            # w loads (appended after qk fillers so ring deps stay behind)
            slot_fillers[0].append(lambda: qk_w_load(1))
            slot_fillers[4].append(lambda: qk_w_load(2))
            slot_fillers[5].append(lambda: qk_w_load(3))
            # proj of block T rides two slots after (3,T); blocks 2 and 3
            # drain in the tail, block 2 first so its ready matmuls hide
            # block 3's l -> 1/l -> rl chain.
            for T, s in ((0, 11), (1, 13)):
                for tp in range(4 * T, 4 * (T + 1)):
                    slot_fillers[s].append(lambda a=tp: proj_tile(a))

            qk_w_load(0)
            qk_dest(0, 0, "q")
            qk_dest(0, 0, "k")
            for s, (j, t) in enumerate(A):
                attn_tile(j, t, fillers=slot_fillers[s])
                if j == NJ - 1:
                    l_recip(t)
                    rl_build(t)
            for tp in range(4 * (NT4 - 2), 4 * NT4):
                proj_tile(tp)
    nc.compile()
    return nc


_NC_CACHE = {}


def _get_nc(S=2048):
    if S not in _NC_CACHE:
        _NC_CACHE[S] = build_nc(S)
    return _NC_CACHE[S]


def make_masks():
    i = np.arange(P)[:, None]
    j2 = np.arange(SQT)[None, :]
    mk = np.zeros((P, 4, SQT), dtype=np.float32)
    for m in range(4):
        mk[:, m, :] = (j2 >= P * m + i).astype(np.float32)
    return mk


def shard_inputs(x, w_qkv, w_proj):
    bf = ml_dtypes.bfloat16
    mk = make_masks().astype(bf)
    ins = []
    for c in range(8):
        b, g = divmod(c, 2)
        ins.append(
            {
                "xT": np.ascontiguousarray(x[b].T).astype(bf),
                "wq": np.ascontiguousarray(
                    w_qkv[:, g * GCOLS : (g + 1) * GCOLS]
                ).astype(bf),
                "wk": np.ascontiguousarray(
                    w_qkv[:, D + g * GCOLS : D + (g + 1) * GCOLS]
                ).astype(bf),
                "wv": np.ascontiguousarray(
                    w_qkv[:, 2 * D + g * GCOLS : 2 * D + (g + 1) * GCOLS]
                ).astype(bf),
                "wp": np.ascontiguousarray(
                    w_proj[g * GCOLS : (g + 1) * GCOLS, :]
                ).astype(bf),
                "mk": mk,
            }
        )
    return ins


_LAST_RESULT = None


def kernel(x, w_qkv, w_proj):
    global _LAST_RESULT
    x = np.asarray(x, dtype=np.float32)
    w_qkv = np.asarray(w_qkv, dtype=np.float32)
    w_proj = np.asarray(w_proj, dtype=np.float32)
    S = x.shape[1]
    nc = _get_nc(S)
    ins = shard_inputs(x, w_qkv, w_proj)
    res = run_bass_kernel_spmd(
        nc,
        ins,
        core_ids=list(range(8)),
        trace=TRACE,
        **TRACE_KWARGS,
    )
    _LAST_RESULT = res
    outs = [res.results[c]["out"].astype(np.float32) for c in range(8)]
    return np.stack([outs[2 * b] + outs[2 * b + 1] for b in range(4)])
